# revision 2
# baseline (speedup 1.0000x reference)
"""Trainium2 Bass kernel for nn_EnsembleNet (10-head MLP ensemble).

Math (per head h):
  h1 = relu(x @ W1[h] + b1[h])      x: [B, 129], W1: [129, 16]
  h2 = relu(h1 @ W2[h] + b2[h])     W2: [16, 16]
  out[h] = h2 @ W3[h] + b3[h]       W3: [16, 16] -> [10, B, 16]

Strategy (data parallel over 8 cores, B=500000 -> 62500/core, padded to
63488 = 31 super-tiles x 2048 samples):
  - Host pre-transposes the batch shard to xT=[128 feat, PAD] bf16 and
    splits off feature 128 as xl=[1, PAD] bf16, so the kernel does zero
    on-device transposes and the 129-feature contraction is one K=128
    bf16 matmul plus one K=1 accumulate per 512-sample tile.
  - All compute stays in the transposed domain [feat/hid, batch]; the
    host undoes the layout when assembling the final [10, B, 16] output.
  - Heads 0-7 ("A", 8*16=128 rows): chained block-diagonal matmuls
    (L1 bf16, L2/L3 float32r - same 1 cyc/row PE rate, better accuracy),
    relu+bias on ACT/DVE, output bias-add written directly as bf16.
  - Heads 8-9 ("B", 32 rows) are packed 4 sub-tiles deep on the
    partition axis (4 x 32 = 128) per super-tile; layer 1 places each
    sub-tile's strip via zero-padded M=128 weight variants accumulating
    into one packed psum bank.
  - Elementwise psum->sbuf ops are split ACT (4x h1 relu, 4x out
    bias-add via Identity) / DVE (4x h2 relu, B-group ops) per super to
    keep both under the PE's ~5.5us/super.
  - One input DMA and two output DMAs per super-tile (4KB+4KB+1KB per
    partition, contiguous), all bf16, issued on the SP HWDGE ring.
"""

import numpy as np

from contextlib import ExitStack

import concourse.bass as bass  # noqa: F401  (AP types come through tile)
import concourse.mybir as mybir
import concourse.tile as tile
from concourse import bacc
from concourse.bass_utils import run_bass_kernel_spmd

F32 = mybir.dt.float32
F32R = mybir.dt.float32r
BF16 = mybir.dt.bfloat16

N_CORES = 8
BATCH = 500000
SHARD = BATCH // N_CORES  # 62500
TILE = 512
SUPER = 4 * TILE  # 2048
N_TILES = 124
PAD = N_TILES * TILE  # 63488
N_SUPERS = N_TILES // 4  # 31

NHEADS = 10
HID = 16
SKIP = 16
IN_DIM = 129


def _block_diag(mats):
    n = len(mats)
    r, c = mats[0].shape
    out = np.zeros((n * r, n * c), dtype=mats[0].dtype)
    for i, m in enumerate(mats):
        out[i * r : (i + 1) * r, i * c : (i + 1) * c] = m
    return out


def _pack_weights(W1, b1, W2, b2, W3, b3):
    """Host-side packing into the SBUF layouts the kernel expects."""
    import ml_dtypes

    BF = ml_dtypes.bfloat16
    W1 = np.asarray(W1, np.float32)
    W2 = np.asarray(W2, np.float32)
    W3 = np.asarray(W3, np.float32)
    b1 = np.asarray(b1, np.float32)
    b2 = np.asarray(b2, np.float32)
    b3 = np.asarray(b3, np.float32)

    d = {}
    # L1 A: lhsT [K=128 feat, M=128 (h,o)], bf16
    d["w1a"] = np.ascontiguousarray(
        W1[:8, :128, :].transpose(1, 0, 2).reshape(128, 128)
    ).astype(BF)
    d["wla"] = np.ascontiguousarray(W1[:8, 128, :].reshape(1, 128)).astype(BF)
    # L1 B: zero-padded M=128 variants, chunk c owns columns c*32..(c+1)*32
    w1b32 = W1[8:, :128, :].transpose(1, 0, 2).reshape(128, 32)
    wlb32 = W1[8:, 128, :].reshape(32)
    w1bs = np.zeros((4, 128, 128), np.float32)
    wlbs = np.zeros((4, 1, 128), np.float32)
    for c in range(4):
        w1bs[c, :, c * 32 : (c + 1) * 32] = w1b32
        wlbs[c, 0, c * 32 : (c + 1) * 32] = wlb32
    d["w1bs"] = w1bs.astype(BF)
    d["wlbs"] = wlbs.astype(BF)
    # L2/L3: block diag [in (h,i), out (h,o)], kept f32 (f32r on device)
    d["w2a"] = _block_diag([W2[h] for h in range(8)])
    w2b1 = _block_diag([W2[8], W2[9]])  # [32, 32]
    d["w2b"] = _block_diag([w2b1] * 4)  # [128, 128] over (c, g)
    d["w3a"] = _block_diag([W3[h] for h in range(8)])
    w3b1 = _block_diag([W3[8], W3[9]])
    d["w3b"] = _block_diag([w3b1] * 4)
    # biases, per-partition [128, 1] f32
    d["b1a"] = b1[:8].reshape(128, 1).copy()
    d["b1b"] = np.tile(b1[8:].reshape(-1), 4).reshape(128, 1)
    d["b2a"] = b2[:8].reshape(128, 1).copy()
    d["b2b"] = np.tile(b2[8:].reshape(-1), 4).reshape(128, 1)
    d["b3a"] = b3[:8].reshape(128, 1).copy()
    d["b3b"] = np.tile(b3[8:].reshape(-1), 4).reshape(128, 1)
    return {k: np.ascontiguousarray(v) for k, v in d.items()}


def _kernel_body(tc, outs, ins, repeat=1):
    nc = tc.nc
    relu = mybir.ActivationFunctionType.Relu
    ident = mybir.ActivationFunctionType.Identity
    add = mybir.AluOpType.add
    amax = mybir.AluOpType.max
    outA, outB = outs["outA"], outs["outB"]
    xT, xl = ins["xT"], ins["xl"]

    with ExitStack() as ctx:
        const = ctx.enter_context(tc.tile_pool(name="const", bufs=1))

        def ld(name, shape, dt=F32):
            t = const.tile(shape, dt, name=name)
            nc.sync.dma_start(t, ins[name])
            return t

        w1a = ld("w1a", [128, 128], BF16)
        wla = ld("wla", [1, 128], BF16)
        w1bs = [None] * 4
        wlbs = [None] * 4
        for c in range(4):
            w1bs[c] = const.tile([128, 128], BF16, name=f"w1bs{c}")
            nc.sync.dma_start(w1bs[c], ins["w1bs"][c])
            wlbs[c] = const.tile([1, 128], BF16, name=f"wlbs{c}")
            nc.sync.dma_start(wlbs[c], ins["wlbs"][c])
        w2a = ld("w2a", [128, 128], F32R)
        w2b = ld("w2b", [128, 128], F32R)
        w3a = ld("w3a", [128, 128], F32R)
        w3b = ld("w3b", [128, 128], F32R)
        b1a = ld("b1a", [128, 1])
        b1b = ld("b1b", [128, 1])
        b2a = ld("b2a", [128, 1])
        b2b = ld("b2b", [128, 1])
        b3a = ld("b3a", [128, 1])
        b3b = ld("b3b", [128, 1])
        # whole-shard last-feature row (124 KB on partition 0)
        xlt = const.tile([1, PAD], BF16, name="xlt")
        nc.sync.dma_start(xlt, xl)

        xt_pool = ctx.enter_context(tc.tile_pool(name="xt", bufs=3))
        h_pool = ctx.enter_context(tc.tile_pool(name="h", bufs=4))
        bsb_pool = ctx.enter_context(tc.tile_pool(name="bsb", bufs=2))
        oa_pool = ctx.enter_context(tc.tile_pool(name="oa", bufs=2))
        ob_pool = ctx.enter_context(tc.tile_pool(name="ob", bufs=2))
        pa1pool = ctx.enter_context(tc.tile_pool(name="pa1", space="PSUM", bufs=2))
        pa2pool = ctx.enter_context(tc.tile_pool(name="pa2", space="PSUM", bufs=2))
        pa3pool = ctx.enter_context(tc.tile_pool(name="pa3", space="PSUM", bufs=2))
        pbpool = ctx.enter_context(tc.tile_pool(name="pb", space="PSUM", bufs=2))

        if repeat > 1:
            # timing-only variant: run the whole body `repeat` times on
            # device so single-dispatch wall time isolates device exec
            ctx.enter_context(tc.For_i(0, repeat, 1))

        for s in range(N_SUPERS):
            xt = xt_pool.tile([128, SUPER], BF16, tag="xt")
            nc.sync.dma_start(xt, xT[:, s * SUPER : (s + 1) * SUPER])
            outa = oa_pool.tile([128, SUPER], BF16, tag="oa")
            pb1 = pbpool.tile([128, TILE], F32, tag="pb")
            for c in range(4):
                t = s * 4 + c
                xc = xt[:, c * TILE : (c + 1) * TILE]
                xlc = xlt[:, t * TILE : (t + 1) * TILE]

                pa1 = pa1pool.tile([128, TILE], F32, tag="pa1")
                nc.tensor.matmul(pa1, w1a, xc, start=True, stop=False)
                nc.tensor.matmul(pa1, wla, xlc, start=False, stop=True)
                h1 = h_pool.tile([128, TILE], F32R, tag="h1")
                nc.scalar.activation(h1, pa1, relu, bias=b1a)

                pa2 = pa2pool.tile([128, TILE], F32, tag="pa2")
                nc.tensor.matmul(pa2, w2a, h1, start=True, stop=True)
                h2 = h_pool.tile([128, TILE], F32R, tag="h2")
                nc.vector.tensor_scalar(h2, pa2, b2a, 0.0, op0=add, op1=amax)

                pa3 = pa3pool.tile([128, TILE], F32, tag="pa3")
                nc.tensor.matmul(pa3, w3a, h2, start=True, stop=True)
                nc.scalar.activation(
                    outa[:, c * TILE : (c + 1) * TILE], pa3, ident, bias=b3a
                )

                # B group layer 1 accumulates all 4 chunks into packed pb1
                nc.tensor.matmul(pb1, w1bs[c], xc, start=(c == 0), stop=False)
                nc.tensor.matmul(pb1, wlbs[c], xlc, start=False, stop=(c == 3))

            # B group layers 2..3, packed [128=(c,g,i), 512]
            h1b = bsb_pool.tile([128, TILE], F32R, tag="h1b")
            nc.vector.tensor_scalar(h1b, pb1, b1b, 0.0, op0=add, op1=amax)
            pb2 = pbpool.tile([128, TILE], F32, tag="pb")
            nc.tensor.matmul(pb2, w2b, h1b, start=True, stop=True)
            h2b = bsb_pool.tile([128, TILE], F32R, tag="h2b")
            nc.vector.tensor_scalar(h2b, pb2, b2b, 0.0, op0=add, op1=amax)
            pb3 = pbpool.tile([128, TILE], F32, tag="pb")
            nc.tensor.matmul(pb3, w3b, h2b, start=True, stop=True)
            outb = ob_pool.tile([128, TILE], BF16, tag="ob")
            nc.vector.tensor_scalar_add(outb, pb3, b3b)

            nc.sync.dma_start(outA[s], outa)
            nc.sync.dma_start(outB[s], outb)


def _make_in_maps(x, W1, b1, W2, b2, W3, b3):
    """Per-core input maps (host-side shard + pack)."""
    import ml_dtypes

    BF = ml_dtypes.bfloat16
    wp = _pack_weights(W1, b1, W2, b2, W3, b3)
    x3 = np.asarray(x, np.float32).reshape(N_CORES, SHARD, IN_DIM)
    in_maps = []
    for c in range(N_CORES):
        xT = np.zeros((128, PAD), BF)
        xT[:, :SHARD] = x3[c, :, :128].T
        xl = np.zeros((1, PAD), BF)
        xl[0, :SHARD] = x3[c, :, 128]
        m = {"xT": xT, "xl": xl}
        m.update(wp)
        in_maps.append(m)
    return in_maps


_CACHE = {}


def _build(repeat=1):
    if repeat in _CACHE:
        return _CACHE[repeat]
    nc = bacc.Bacc(
        "TRN2",
        target_bir_lowering=False,
        debug=False,
        num_devices=N_CORES,
    )
    ins = {}
    ins["xT"] = nc.dram_tensor("xT", (128, PAD), BF16, kind="ExternalInput").ap()
    ins["xl"] = nc.dram_tensor("xl", (1, PAD), BF16, kind="ExternalInput").ap()
    for name, shape, dt in [
        ("w1a", (128, 128), BF16),
        ("wla", (1, 128), BF16),
        ("w1bs", (4, 128, 128), BF16),
        ("wlbs", (4, 1, 128), BF16),
        ("w2a", (128, 128), F32R),
        ("w2b", (128, 128), F32R),
        ("w3a", (128, 128), F32R),
        ("w3b", (128, 128), F32R),
        ("b1a", (128, 1), F32),
        ("b1b", (128, 1), F32),
        ("b2a", (128, 1), F32),
        ("b2b", (128, 1), F32),
        ("b3a", (128, 1), F32),
        ("b3b", (128, 1), F32),
    ]:
        ins[name] = nc.dram_tensor(name, shape, dt, kind="ExternalInput").ap()
    outs = {
        "outA": nc.dram_tensor(
            "outA", (N_SUPERS, 128, SUPER), BF16, kind="ExternalOutput"
        ).ap(),
        "outB": nc.dram_tensor(
            "outB", (N_SUPERS, 128, TILE), BF16, kind="ExternalOutput"
        ).ap(),
    }
    with tile.TileContext(nc) as tc:
        _kernel_body(tc, outs, ins, repeat=repeat)
    nc.compile()
    _CACHE[repeat] = nc
    return nc


def _decode_out(results):
    """Device layouts -> [10, BATCH, 16] f32."""
    out = np.empty((NHEADS, BATCH, SKIP), np.float32)
    for c in range(N_CORES):
        oa = np.asarray(results[c]["outA"])  # [31, 128, 2048] bf16
        ob = np.asarray(results[c]["outB"])  # [31, 128, 512] bf16
        # A: [s, (h, o), (j)] -> sample = s*2048 + j
        a = oa.reshape(N_SUPERS, 8, SKIP, SUPER).transpose(1, 0, 3, 2)
        a = a.reshape(8, PAD, SKIP)
        out[:8, c * SHARD : (c + 1) * SHARD] = a[:, :SHARD]
        # B: [s, (cc, g, o), b] -> sample = s*2048 + cc*512 + b
        b = ob.reshape(N_SUPERS, 4, 2, SKIP, TILE).transpose(2, 0, 1, 4, 3)
        b = b.reshape(2, PAD, SKIP)
        out[8:, c * SHARD : (c + 1) * SHARD] = b[:, :SHARD]
    return out


def kernel(x, W1, b1, W2, b2, W3, b3, _want_trace=False):
    in_maps = _make_in_maps(x, W1, b1, W2, b2, W3, b3)
    nc = _build()
    res = run_bass_kernel_spmd(
        nc, in_maps, core_ids=list(range(N_CORES)), trace=_want_trace
    )
    if _want_trace:
        kernel.last_results = res
    return _decode_out(res.results)


# revision 8
# speedup vs baseline: 2.2664x; 2.2664x over previous
"""Trainium2 Bass kernel for nn_EnsembleNet (10-head MLP ensemble).

Math (per head h):
  h1 = relu(x @ W1[h] + b1[h])      x: [B, 129], W1: [129, 16]
  h2 = relu(h1 @ W2[h] + b2[h])     W2: [16, 16]
  out[h] = h2 @ W3[h] + b3[h]       W3: [16, 16] -> [10, B, 16]

Strategy (data parallel over 8 cores, B=500000 -> 62500/core, padded to
63488 = 31 super-tiles x 2048 samples):
  - Host pre-transposes the batch shard to xT=[128 feat, PAD] bf16 and
    splits off feature 128 as xl=[1, PAD] bf16, so the kernel does zero
    on-device transposes and the 129-feature contraction is one K=128
    bf16 matmul plus one K=1 accumulate per 512-sample tile.
  - All compute stays in the transposed domain [feat/hid, batch]; the
    host undoes the layout when assembling the final [10, B, 16] output.
  - Heads 0-7 ("A", 8*16=128 rows): chained block-diagonal matmuls
    (L1 bf16, L2/L3 float32r - same 1 cyc/row PE rate, better accuracy),
    relu+bias on ACT/DVE, output bias-add written directly as bf16.
  - Heads 8-9 ("B", 32 rows) are packed 4 sub-tiles deep on the
    partition axis (4 x 32 = 128) per super-tile; layer 1 places each
    sub-tile's strip via zero-padded M=128 weight variants accumulating
    into one packed psum bank.
  - Elementwise psum->sbuf ops are split ACT (4x h1 relu, 4x out
    bias-add via Identity) / DVE (4x h2 relu, B-group ops) per super to
    keep both under the PE's ~5.5us/super.
  - One input DMA and two output DMAs per super-tile (4KB+4KB+1KB per
    partition, contiguous), all bf16, issued on the SP HWDGE ring.
"""

import os

import numpy as np

from contextlib import ExitStack

import concourse.bass as bass  # noqa: F401  (AP types come through tile)
import concourse.mybir as mybir
import concourse.tile as tile
from concourse import bacc
from concourse.bass_utils import run_bass_kernel_spmd

F32 = mybir.dt.float32
F32R = mybir.dt.float32r
BF16 = mybir.dt.bfloat16

N_CORES = 8
BATCH = 500000
SHARD = BATCH // N_CORES  # 62500
TILE = 512
SUPER = 4 * TILE  # 2048
N_TILES = 124
PAD = N_TILES * TILE  # 63488
N_SUPERS = N_TILES // 4  # 31

NHEADS = 10
HID = 16
SKIP = 16
IN_DIM = 129


def _block_diag(mats):
    n = len(mats)
    r, c = mats[0].shape
    out = np.zeros((n * r, n * c), dtype=mats[0].dtype)
    for i, m in enumerate(mats):
        out[i * r : (i + 1) * r, i * c : (i + 1) * c] = m
    return out


def _pack_weights(W1, b1, W2, b2, W3, b3):
    """Host-side packing into the SBUF layouts the kernel expects."""
    import ml_dtypes

    BF = ml_dtypes.bfloat16
    W1 = np.asarray(W1, np.float32)
    W2 = np.asarray(W2, np.float32)
    W3 = np.asarray(W3, np.float32)
    b1 = np.asarray(b1, np.float32)
    b2 = np.asarray(b2, np.float32)
    b3 = np.asarray(b3, np.float32)

    d = {}
    # L1 A: lhsT [K=128 feat, M=128 (h,o)], bf16
    d["w1a"] = np.ascontiguousarray(
        W1[:8, :128, :].transpose(1, 0, 2).reshape(128, 128)
    ).astype(BF)
    d["wla"] = np.ascontiguousarray(W1[:8, 128, :].reshape(1, 128)).astype(BF)
    # L1 B: zero-padded M=128 variants, chunk c owns columns c*32..(c+1)*32
    w1b32 = W1[8:, :128, :].transpose(1, 0, 2).reshape(128, 32)
    wlb32 = W1[8:, 128, :].reshape(32)
    w1bs = np.zeros((4, 128, 128), np.float32)
    wlbs = np.zeros((4, 1, 128), np.float32)
    for c in range(4):
        w1bs[c, :, c * 32 : (c + 1) * 32] = w1b32
        wlbs[c, 0, c * 32 : (c + 1) * 32] = wlb32
    d["w1bs"] = w1bs.astype(BF)
    d["wlbs"] = wlbs.astype(BF)
    # L2/L3: block diag [in (h,i), out (h,o)], kept f32 (f32r on device)
    d["w2a"] = _block_diag([W2[h] for h in range(8)])
    w2b1 = _block_diag([W2[8], W2[9]])  # [32, 32]
    d["w2b"] = _block_diag([w2b1] * 4)  # [128, 128] over (c, g)
    d["w3a"] = _block_diag([W3[h] for h in range(8)])
    w3b1 = _block_diag([W3[8], W3[9]])
    d["w3b"] = _block_diag([w3b1] * 4)
    # biases, per-partition [128, 1] f32
    d["b1a"] = b1[:8].reshape(128, 1).copy()
    d["b1b"] = np.tile(b1[8:].reshape(-1), 4).reshape(128, 1)
    d["b2a"] = b2[:8].reshape(128, 1).copy()
    d["b2b"] = np.tile(b2[8:].reshape(-1), 4).reshape(128, 1)
    d["b3a"] = b3[:8].reshape(128, 1).copy()
    d["b3b"] = np.tile(b3[8:].reshape(-1), 4).reshape(128, 1)
    return {k: np.ascontiguousarray(v) for k, v in d.items()}


def _kernel_body(tc, outs, ins, repeat=1):
    nc = tc.nc
    relu = mybir.ActivationFunctionType.Relu
    ident = mybir.ActivationFunctionType.Identity
    add = mybir.AluOpType.add
    amax = mybir.AluOpType.max
    outA, outB = outs["outA"], outs["outB"]
    xT, xl = ins["xT"], ins["xl"]

    with ExitStack() as ctx:
        const = ctx.enter_context(tc.tile_pool(name="const", bufs=1))

        def ld(name, shape, dt=F32):
            t = const.tile(shape, dt, name=name)
            nc.sync.dma_start(t, ins[name])
            return t

        w1a = ld("w1a", [128, 128], BF16)
        wla = ld("wla", [1, 128], BF16)
        w1bs = [None] * 4
        wlbs = [None] * 4
        for c in range(4):
            w1bs[c] = const.tile([128, 128], BF16, name=f"w1bs{c}")
            nc.sync.dma_start(w1bs[c], ins["w1bs"][c])
            wlbs[c] = const.tile([1, 128], BF16, name=f"wlbs{c}")
            nc.sync.dma_start(wlbs[c], ins["wlbs"][c])
        w2a = ld("w2a", [128, 128], F32R)
        w2b = ld("w2b", [128, 128], F32R)
        w3a = ld("w3a", [128, 128], F32R)
        w3b = ld("w3b", [128, 128], F32R)
        b1a = ld("b1a", [128, 1])
        b1b = ld("b1b", [128, 1])
        b2a = ld("b2a", [128, 1])
        b2b = ld("b2b", [128, 1])
        b3a = ld("b3a", [128, 1])
        b3b = ld("b3b", [128, 1])
        # whole-shard last-feature row (124 KB on partition 0)
        xlt = const.tile([1, PAD], BF16, name="xlt")
        nc.sync.dma_start(xlt, xl)

        xt_pool = ctx.enter_context(tc.tile_pool(name="xt", bufs=3))
        h_pool = ctx.enter_context(tc.tile_pool(name="h", bufs=4))
        bsb_pool = ctx.enter_context(tc.tile_pool(name="bsb", bufs=2))
        oa_pool = ctx.enter_context(tc.tile_pool(name="oa", bufs=2))
        ob_pool = ctx.enter_context(tc.tile_pool(name="ob", bufs=2))
        pa1pool = ctx.enter_context(tc.tile_pool(name="pa1", space="PSUM", bufs=2))
        pa2pool = ctx.enter_context(tc.tile_pool(name="pa2", space="PSUM", bufs=2))
        pa3pool = ctx.enter_context(tc.tile_pool(name="pa3", space="PSUM", bufs=2))
        pbpool = ctx.enter_context(tc.tile_pool(name="pb", space="PSUM", bufs=2))

        if repeat > 1:
            # timing-only variant: run the whole body `repeat` times on
            # device so single-dispatch wall time isolates device exec
            ctx.enter_context(tc.For_i(0, repeat, 1))

        strip = os.environ.get("K_STRIP", "")
        if strip == "dmaonly":
            # loop moves the same bytes with no compute: isolates the
            # memory system / DMA path
            for s in range(N_SUPERS):
                xt = xt_pool.tile([128, SUPER], BF16, tag="xt")
                nc.sync.dma_start(xt, xT[:, s * SUPER : (s + 1) * SUPER])
                nc.sync.dma_start(outA[s], xt)
                nc.sync.dma_start(outB[s], xt[:, :TILE])
            return
        if strip == "peonly":
            # loop does all matmuls + elementwise on one preloaded
            # super-tile: isolates the compute pipeline
            xt = xt_pool.tile([128, SUPER], BF16, tag="xt")
            nc.sync.dma_start(xt, xT[:, :SUPER])
            outa = oa_pool.tile([128, SUPER], BF16, tag="oa")
            for s in range(N_SUPERS):
                pb1 = pbpool.tile([128, TILE], F32, tag="pb")
                for c in range(4):
                    xc = xt[:, c * TILE : (c + 1) * TILE]
                    xlc = xlt[:, c * TILE : (c + 1) * TILE]
                    pa1 = pa1pool.tile([128, TILE], F32, tag="pa1")
                    nc.tensor.matmul(pa1, w1a, xc, start=True, stop=False)
                    nc.tensor.matmul(pa1, wla, xlc, start=False, stop=True)
                    h1 = h_pool.tile([128, TILE], F32R, tag="h1")
                    nc.scalar.activation(h1, pa1, relu, bias=b1a)
                    pa2 = pa2pool.tile([128, TILE], F32, tag="pa2")
                    nc.tensor.matmul(pa2, w2a, h1, start=True, stop=True)
                    h2 = h_pool.tile([128, TILE], F32R, tag="h2")
                    nc.vector.tensor_scalar(h2, pa2, b2a, 0.0, op0=add, op1=amax)
                    pa3 = pa3pool.tile([128, TILE], F32, tag="pa3")
                    nc.tensor.matmul(pa3, w3a, h2, start=True, stop=True)
                    nc.scalar.activation(
                        outa[:, c * TILE : (c + 1) * TILE], pa3, ident, bias=b3a
                    )
                    nc.tensor.matmul(pb1, w1bs[c], xc, start=(c == 0), stop=False)
                    nc.tensor.matmul(pb1, wlbs[c], xlc, start=False, stop=(c == 3))
                h1b = bsb_pool.tile([128, TILE], F32R, tag="h1b")
                nc.vector.tensor_scalar(h1b, pb1, b1b, 0.0, op0=add, op1=amax)
                pb2 = pbpool.tile([128, TILE], F32, tag="pb")
                nc.tensor.matmul(pb2, w2b, h1b, start=True, stop=True)
                h2b = bsb_pool.tile([128, TILE], F32R, tag="h2b")
                nc.vector.tensor_scalar(h2b, pb2, b2b, 0.0, op0=add, op1=amax)
                pb3 = pbpool.tile([128, TILE], F32, tag="pb")
                nc.tensor.matmul(pb3, w3b, h2b, start=True, stop=True)
                outb = ob_pool.tile([128, TILE], BF16, tag="ob")
                nc.vector.tensor_scalar_add(outb, pb3, b3b)
            nc.sync.dma_start(outA[0], outa)
            nc.sync.dma_start(outB[0], outb)
            return

        # Software pipeline over "slot tiles": per super, 4 A-tiles (heads
        # 0-7, 512 samples each) + 1 B-tile (heads 8-9, packed (c,g,o) over
        # the whole super). Engines on TRN2 are strict-FIFO, so each stage
        # of a tile is emitted one slot after the stage it depends on —
        # every engine's queue head is always ready and no head-of-line
        # blocking occurs. Stage s of tile i executes in slot i+s:
        #   slot i: L1 matmuls(i), h1(i-1), L2(i-2), h2(i-3), L3(i-4),
        #           out(i-5), out-DMA(i-6)
        tiles = []
        for s in range(N_SUPERS):
            tiles.extend([("A", s, c) for c in range(4)])
            tiles.append(("B", s, 0))
        NT = len(tiles)

        xts, pb1s, outas, obs = {}, {}, {}, {}
        pa1s, pa2s, pa3s, h1s, h2s = {}, {}, {}, {}, {}

        def in_dma(s):
            if s < N_SUPERS:
                xts[s] = xt_pool.tile([128, SUPER], BF16, tag="xt", name=f"xt{s}")
                nc.sync.dma_start(xts[s], xT[:, s * SUPER : (s + 1) * SUPER])

        def stage1(i):
            kind, s, c = tiles[i]
            if kind == "A":
                if c == 0:
                    in_dma(s + 2)
                xc = xts[s][:, c * TILE : (c + 1) * TILE]
                t = s * 4 + c
                xlc = xlt[:, t * TILE : (t + 1) * TILE]
                pa1 = pa1pool.tile([128, TILE], F32, tag="pa1", name=f"pa1_{i}")
                nc.tensor.matmul(pa1, w1a, xc, start=True, stop=False)
                nc.tensor.matmul(pa1, wla, xlc, start=False, stop=True)
                pa1s[i] = pa1
                if c == 0:
                    pb1s[s] = pbpool.tile([128, TILE], F32, tag="pb1", name=f"pb1_{s}")
                nc.tensor.matmul(pb1s[s], w1bs[c], xc, start=(c == 0), stop=False)
                nc.tensor.matmul(pb1s[s], wlbs[c], xlc, start=False, stop=(c == 3))
            else:
                pa1s[i] = pb1s[s]

        def stage2(i):
            kind, s, c = tiles[i]
            h1 = h_pool.tile([128, TILE], F32R, tag="h1", name=f"h1_{i}")
            if kind == "A":
                nc.scalar.activation(h1, pa1s.pop(i), relu, bias=b1a)
            else:
                nc.vector.tensor_scalar(
                    h1, pa1s.pop(i), b1b, 0.0, op0=add, op1=amax
                )
            h1s[i] = h1

        def stage3(i):
            kind = tiles[i][0]
            pa2 = pa2pool.tile([128, TILE], F32, tag="pa2", name=f"pa2_{i}")
            nc.tensor.matmul(
                pa2, w2a if kind == "A" else w2b, h1s.pop(i), start=True, stop=True
            )
            pa2s[i] = pa2

        def stage4(i):
            kind = tiles[i][0]
            h2 = h_pool.tile([128, TILE], F32R, tag="h2", name=f"h2_{i}")
            nc.vector.tensor_scalar(
                h2, pa2s.pop(i), b2a if kind == "A" else b2b, 0.0, op0=add, op1=amax
            )
            h2s[i] = h2

        def stage5(i):
            kind = tiles[i][0]
            pa3 = pa3pool.tile([128, TILE], F32, tag="pa3", name=f"pa3_{i}")
            nc.tensor.matmul(
                pa3, w3a if kind == "A" else w3b, h2s.pop(i), start=True, stop=True
            )
            pa3s[i] = pa3

        def stage6(i):
            kind, s, c = tiles[i]
            if kind == "A":
                if c == 0:
                    outas[s] = oa_pool.tile([128, SUPER], BF16, tag="oa", name=f"oa{s}")
                nc.scalar.activation(
                    outas[s][:, c * TILE : (c + 1) * TILE],
                    pa3s.pop(i),
                    ident,
                    bias=b3a,
                )
            else:
                obs[s] = ob_pool.tile([128, TILE], BF16, tag="ob", name=f"ob{s}")
                nc.vector.tensor_scalar_add(obs[s], pa3s.pop(i), b3b)

        def stage7(i):
            kind, s, c = tiles[i]
            if kind == "A" and c == 3:
                nc.sync.dma_start(outA[s], outas.pop(s))
            elif kind == "B":
                nc.sync.dma_start(outB[s], obs.pop(s))

        in_dma(0)
        in_dma(1)
        stages = [stage7, stage6, stage5, stage4, stage3, stage2, stage1]
        for slot in range(NT + 6):
            for d, fn in enumerate(stages):
                i = slot - 6 + d
                if 0 <= i < NT:
                    fn(i)


def _make_in_maps(x, W1, b1, W2, b2, W3, b3):
    """Per-core input maps (host-side shard + pack)."""
    import ml_dtypes

    BF = ml_dtypes.bfloat16
    wp = _pack_weights(W1, b1, W2, b2, W3, b3)
    x3 = np.asarray(x, np.float32).reshape(N_CORES, SHARD, IN_DIM)
    in_maps = []
    for c in range(N_CORES):
        xT = np.zeros((128, PAD), BF)
        xT[:, :SHARD] = x3[c, :, :128].T
        xl = np.zeros((1, PAD), BF)
        xl[0, :SHARD] = x3[c, :, 128]
        m = {"xT": xT, "xl": xl}
        m.update(wp)
        in_maps.append(m)
    return in_maps


_CACHE = {}


def _build(repeat=1):
    key = (repeat, os.environ.get("K_STRIP", ""))
    if key in _CACHE:
        return _CACHE[key]
    nc = bacc.Bacc(
        "TRN2",
        target_bir_lowering=False,
        debug=False,
        num_devices=N_CORES,
    )
    ins = {}
    ins["xT"] = nc.dram_tensor("xT", (128, PAD), BF16, kind="ExternalInput").ap()
    ins["xl"] = nc.dram_tensor("xl", (1, PAD), BF16, kind="ExternalInput").ap()
    for name, shape, dt in [
        ("w1a", (128, 128), BF16),
        ("wla", (1, 128), BF16),
        ("w1bs", (4, 128, 128), BF16),
        ("wlbs", (4, 1, 128), BF16),
        ("w2a", (128, 128), F32R),
        ("w2b", (128, 128), F32R),
        ("w3a", (128, 128), F32R),
        ("w3b", (128, 128), F32R),
        ("b1a", (128, 1), F32),
        ("b1b", (128, 1), F32),
        ("b2a", (128, 1), F32),
        ("b2b", (128, 1), F32),
        ("b3a", (128, 1), F32),
        ("b3b", (128, 1), F32),
    ]:
        ins[name] = nc.dram_tensor(name, shape, dt, kind="ExternalInput").ap()
    outs = {
        "outA": nc.dram_tensor(
            "outA", (N_SUPERS, 128, SUPER), BF16, kind="ExternalOutput"
        ).ap(),
        "outB": nc.dram_tensor(
            "outB", (N_SUPERS, 128, TILE), BF16, kind="ExternalOutput"
        ).ap(),
    }
    with tile.TileContext(nc) as tc:
        _kernel_body(tc, outs, ins, repeat=repeat)
    nc.compile()
    _CACHE[key] = nc
    return nc


def _decode_out(results):
    """Device layouts -> [10, BATCH, 16] f32."""
    out = np.empty((NHEADS, BATCH, SKIP), np.float32)
    for c in range(N_CORES):
        oa = np.asarray(results[c]["outA"])  # [31, 128, 2048] bf16
        ob = np.asarray(results[c]["outB"])  # [31, 128, 512] bf16
        # A: [s, (h, o), (j)] -> sample = s*2048 + j
        a = oa.reshape(N_SUPERS, 8, SKIP, SUPER).transpose(1, 0, 3, 2)
        a = a.reshape(8, PAD, SKIP)
        out[:8, c * SHARD : (c + 1) * SHARD] = a[:, :SHARD]
        # B: [s, (cc, g, o), b] -> sample = s*2048 + cc*512 + b
        b = ob.reshape(N_SUPERS, 4, 2, SKIP, TILE).transpose(2, 0, 1, 4, 3)
        b = b.reshape(2, PAD, SKIP)
        out[8:, c * SHARD : (c + 1) * SHARD] = b[:, :SHARD]
    return out


def kernel(x, W1, b1, W2, b2, W3, b3, _want_trace=False):
    in_maps = _make_in_maps(x, W1, b1, W2, b2, W3, b3)
    nc = _build()
    res = run_bass_kernel_spmd(
        nc, in_maps, core_ids=list(range(N_CORES)), trace=_want_trace
    )
    if _want_trace:
        kernel.last_results = res
    return _decode_out(res.results)


# revision 22
# speedup vs baseline: 2.9199x; 1.2884x over previous
"""Trainium2 Bass kernel for nn_EnsembleNet (10-head MLP ensemble).

Math (per head h):
  h1 = relu(x @ W1[h] + b1[h])      x: [B, 129], W1: [129, 16]
  h2 = relu(h1 @ W2[h] + b2[h])     W2: [16, 16]
  out[h] = h2 @ W3[h] + b3[h]       W3: [16, 16] -> [10, B, 16]

Strategy (data parallel over 8 cores, B=500000 -> 62500/core, padded to
63488 = 31 super-tiles x 2048 samples):
  - Host pre-transposes the batch shard to xT=[128 feat, PAD] bf16 and
    splits off feature 128 as xl=[1, PAD] bf16, so the kernel does zero
    on-device transposes and the 129-feature contraction is one K=128
    bf16 matmul plus one K=1 accumulate per 512-sample tile. Everything
    stays in the transposed domain [feat/hid, batch]; the host undoes
    the layout when assembling the final [10, B, 16] f32 output.
  - Per super-tile there are 5 "slot tiles": 4 A-tiles (heads 0-7,
    8*16=128 psum rows, 512 samples) + 1 B-tile (heads 8-9 packed
    (chunk, head, out) = 4*2*16 = 128 rows covering the whole super).
    B layer 1 accumulates chunk strips via zero-padded M=128 weight
    variants during the A slots; all four last-feature rank-1 updates
    collapse into ONE K=4 block-diagonal matmul in the B slot.
  - All matmuls bf16 (1 cyc/row on the PE, lower power than f32r, and
    psum accumulates in f32; scale-relative absmax err 5.8e-3 vs the
    2e-2 budget). 23 PE streams of 512 columns per super-tile.
  - TRN2 engines are strict-FIFO, so the emission is software-pipelined
    over slot tiles: stage k of tile i is emitted in slot i+k (L1 mm ->
    h1 relu -> L2 mm -> h2 relu -> L3 mm -> out bias-add -> DMA). Every
    dependency completes one full slot before its consumer, so no engine
    queue head ever blocks (this alone was worth 2.2x on hardware vs
    chain-order emission).
  - Elementwise psum->sbuf ops split ACT (h1 relu+bias, A out bias-add
    via Identity, both writing bf16) / DVE (h2 relu, B-tile ops) - 8+7
    ops per super, ~matched to the PE's ~4.9us.
  - PSUM: pa1/pa2/pa3/pb1 pools, 2 banks each = all 8 banks, giving
    full double-buffering at every pipeline stage.
  - One input DMA ([128, 4KB]) and one merged output DMA ([128, 5KB],
    A block + B block side by side in outAB) per super-tile, bf16, on
    the SP HWDGE ring.
  Measured on 8 axon trn2 cores (repeat-loop difference method):
  ~190-220 us per pass (machine drifts between runs), down from the
  492 us baseline; rel err 5.785e-3.
"""

import os

import numpy as np

from contextlib import ExitStack

import concourse.bass as bass  # noqa: F401  (AP types come through tile)
import concourse.mybir as mybir
import concourse.tile as tile
from concourse import bacc
from concourse.bass_utils import run_bass_kernel_spmd

F32 = mybir.dt.float32
F32R = mybir.dt.float32r
BF16 = mybir.dt.bfloat16

N_CORES = 8
BATCH = 500000
SHARD = BATCH // N_CORES  # 62500
TILE = 512
SUPER = 4 * TILE  # 2048
N_TILES = 124
PAD = N_TILES * TILE  # 63488
N_SUPERS = N_TILES // 4  # 31

NHEADS = 10
HID = 16
SKIP = 16
IN_DIM = 129


def _block_diag(mats):
    n = len(mats)
    r, c = mats[0].shape
    out = np.zeros((n * r, n * c), dtype=mats[0].dtype)
    for i, m in enumerate(mats):
        out[i * r : (i + 1) * r, i * c : (i + 1) * c] = m
    return out


def _pack_weights(W1, b1, W2, b2, W3, b3):
    """Host-side packing into the SBUF layouts the kernel expects."""
    import ml_dtypes

    BF = ml_dtypes.bfloat16
    W1 = np.asarray(W1, np.float32)
    W2 = np.asarray(W2, np.float32)
    W3 = np.asarray(W3, np.float32)
    b1 = np.asarray(b1, np.float32)
    b2 = np.asarray(b2, np.float32)
    b3 = np.asarray(b3, np.float32)

    d = {}
    # L1 A: lhsT [K=128 feat, M=128 (h,o)], bf16
    d["w1a"] = np.ascontiguousarray(
        W1[:8, :128, :].transpose(1, 0, 2).reshape(128, 128)
    ).astype(BF)
    d["wla"] = np.ascontiguousarray(W1[:8, 128, :].reshape(1, 128)).astype(BF)
    # L1 B: compact [K=128 feat, M=32 (g,o)] + last-feature row; placed
    # per-chunk on the PE via column tiling (tile_position), not padding
    d["w1b"] = np.ascontiguousarray(
        W1[8:, :128, :].transpose(1, 0, 2).reshape(128, 32)
    ).astype(BF)
    d["wlb"] = np.ascontiguousarray(W1[8:, 128, :].reshape(1, 32)).astype(BF)
    # L2/L3: block diag [in (h,i), out (h,o)], bf16 like L1
    l23 = np.float32 if os.environ.get("K_L23", "") == "f32r" else BF
    d["w2a"] = _block_diag([W2[h] for h in range(8)]).astype(l23)
    w2b1 = _block_diag([W2[8], W2[9]])  # [32, 32]
    d["w2b"] = _block_diag([w2b1] * 4).astype(l23)  # [128, 128] over (c, g)
    d["w3a"] = _block_diag([W3[h] for h in range(8)]).astype(l23)
    w3b1 = _block_diag([W3[8], W3[9]])
    d["w3b"] = _block_diag([w3b1] * 4).astype(l23)
    # biases, per-partition [128, 1] f32
    d["b1a"] = b1[:8].reshape(128, 1).copy()
    d["b1b"] = np.tile(b1[8:].reshape(-1), 4).reshape(128, 1)
    d["b2a"] = b2[:8].reshape(128, 1).copy()
    d["b2b"] = np.tile(b2[8:].reshape(-1), 4).reshape(128, 1)
    d["b3a"] = b3[:8].reshape(128, 1).copy()
    d["b3b"] = np.tile(b3[8:].reshape(-1), 4).reshape(128, 1)
    return {k: np.ascontiguousarray(v) for k, v in d.items()}


def _kernel_body(tc, outs, ins, repeat=1):
    nc = tc.nc
    relu = mybir.ActivationFunctionType.Relu
    ident = mybir.ActivationFunctionType.Identity
    add = mybir.AluOpType.add
    amax = mybir.AluOpType.max
    outAB = outs["outAB"]
    xT, xl = ins["xT"], ins["xl"]

    with ExitStack() as ctx:
        const = ctx.enter_context(tc.tile_pool(name="const", bufs=1))

        def ld(name, shape, dt=F32):
            t = const.tile(shape, dt, name=name)
            nc.sync.dma_start(t, ins[name])
            return t

        w1a = ld("w1a", [128, 128], BF16)
        wla = ld("wla", [1, 128], BF16)
        w1b = ld("w1b", [128, 32], BF16)
        # last-feature B weights replicated on partitions 0/32/64/96 for the
        # 32x32-tiled rank-1 accumulates
        wlb4 = const.tile([97, 32], BF16, name="wlb4")
        for c in range(4):
            nc.sync.dma_start(wlb4[32 * c : 32 * c + 1, :], ins["wlb"])
        L23 = F32R if os.environ.get("K_L23", "") == "f32r" else BF16
        w2a = ld("w2a", [128, 128], L23)
        w2b = ld("w2b", [128, 128], L23)
        w3a = ld("w3a", [128, 128], L23)
        w3b = ld("w3b", [128, 128], L23)
        b1a = ld("b1a", [128, 1])
        b1b = ld("b1b", [128, 1])
        b2a = ld("b2a", [128, 1])
        b2b = ld("b2b", [128, 1])
        b3a = ld("b3a", [128, 1])
        b3b = ld("b3b", [128, 1])
        # whole-shard last-feature row, replicated on partitions
        # 0/32/64/96 (for the tiled B rank-1s; A uses row 0)
        xl4t = const.tile([97, PAD], BF16, name="xl4t")
        for c in range(4):
            nc.sync.dma_start(xl4t[32 * c : 32 * c + 1, :], xl)

        xt_pool = ctx.enter_context(tc.tile_pool(name="xt", bufs=3))
        h_pool = ctx.enter_context(tc.tile_pool(name="h", bufs=4))
        oa_pool = ctx.enter_context(tc.tile_pool(name="oa", bufs=2))
        pa1pool = ctx.enter_context(tc.tile_pool(name="pa1", space="PSUM", bufs=3))
        pa2pool = ctx.enter_context(tc.tile_pool(name="pa2", space="PSUM", bufs=2))
        pa3pool = ctx.enter_context(tc.tile_pool(name="pa3", space="PSUM", bufs=3))

        if repeat > 1:
            # timing-only variant: run the whole body `repeat` times on
            # device so single-dispatch wall time isolates device exec
            ctx.enter_context(tc.For_i(0, repeat, 1))

        noxl = os.environ.get("K_NOXL", "") == "1"
        strip = os.environ.get("K_STRIP", "")
        if strip == "dmaonly":
            # loop moves the same bytes with no compute: isolates the
            # memory system / DMA path
            for s in range(N_SUPERS):
                xt = xt_pool.tile([128, SUPER], BF16, tag="xt")
                nc.sync.dma_start(xt, xT[:, s * SUPER : (s + 1) * SUPER])
                nc.sync.dma_start(outAB[s, :, :SUPER], xt)
                nc.sync.dma_start(outAB[s, :, SUPER:], xt[:, :TILE])
            return
        # Software pipeline over "slot tiles": per super, 4 A-tiles (heads
        # 0-7, 512 samples each) + 1 B-tile (heads 8-9, packed (c,g,o) over
        # the whole super). Engines on TRN2 are strict-FIFO, so each stage
        # of a tile is emitted one slot after the stage it depends on —
        # every engine's queue head is always ready and no head-of-line
        # blocking occurs. Stage s of tile i executes in slot i+s:
        #   slot i: L1 matmuls(i), h1(i-1), L2(i-2), h2(i-3), L3(i-4),
        #           out(i-5), out-DMA(i-6)
        tiles = []
        for s in range(N_SUPERS):
            tiles.extend([("A", s, c) for c in range(4)])
            tiles.append(("B", s, 0))
        NT = len(tiles)

        xts, outas = {}, {}
        pa1s, pa2s, pa3s, h1s, h2s = {}, {}, {}, {}, {}

        def in_dma(s):
            if s < N_SUPERS:
                xts[s] = xt_pool.tile([128, SUPER], BF16, tag="xt", name=f"xt{s}")
                nc.sync.dma_start(xts[s], xT[:, s * SUPER : (s + 1) * SUPER])

        def stage1(i):
            kind, s, c = tiles[i]
            if kind == "A":
                if c == 0:
                    in_dma(s + 2)
                xc = xts[s][:, c * TILE : (c + 1) * TILE]
                t = s * 4 + c
                xlc = xl4t[0:1, t * TILE : (t + 1) * TILE]
                pa1 = pa1pool.tile([128, TILE], F32, tag="pa1", name=f"pa1_{i}")
                if noxl:
                    nc.tensor.matmul(pa1, w1a, xc, start=True, stop=True)
                else:
                    nc.tensor.matmul(pa1, w1a, xc, start=True, stop=False)
                    nc.tensor.matmul(pa1, wla, xlc, start=False, stop=True)
                pa1s[i] = pa1
            else:
                # B layer 1: four column-tiled [128,32] matmuls place each
                # chunk's (g,o) strip at psum partitions 32c..32c+32, then
                # four 32x32-tiled K=1 last-feature accumulates run
                # concurrently on the diagonal tiles.
                pb = pa1pool.tile([128, TILE], F32, tag="pa1", name=f"pb_{i}")
                for c4 in range(4):
                    xc = xts[s][:, c4 * TILE : (c4 + 1) * TILE]
                    nc.tensor.matmul(
                        pb[32 * c4 : 32 * (c4 + 1), :],
                        w1b,
                        xc,
                        start=True,
                        stop=noxl,
                        tile_position=(0, 32 * c4),
                    )
                if not noxl:
                    for c4 in range(4):
                        t = s * 4 + c4
                        xlc = xl4t[32 * c4 : 32 * c4 + 1, t * TILE : (t + 1) * TILE]
                        nc.tensor.matmul(
                            pb[32 * c4 : 32 * (c4 + 1), :],
                            wlb4[32 * c4 : 32 * c4 + 1, :],
                            xlc,
                            start=False,
                            stop=True,
                            tile_position=(32 * c4, 32 * c4),
                        )
                pa1s[i] = pb

        def stage2(i):
            kind, s, c = tiles[i]
            h1 = h_pool.tile([128, TILE], L23, tag="h1", name=f"h1_{i}")
            if kind == "A":
                nc.scalar.activation(h1, pa1s.pop(i), relu, bias=b1a)
            else:
                nc.vector.tensor_scalar(
                    h1, pa1s.pop(i), b1b, 0.0, op0=add, op1=amax
                )
            h1s[i] = h1

        def stage3(i):
            kind = tiles[i][0]
            pa2 = pa2pool.tile([128, TILE], F32, tag="pa2", name=f"pa2_{i}")
            nc.tensor.matmul(
                pa2, w2a if kind == "A" else w2b, h1s.pop(i), start=True, stop=True
            )
            pa2s[i] = pa2

        def stage4(i):
            kind = tiles[i][0]
            h2 = h_pool.tile([128, TILE], L23, tag="h2", name=f"h2_{i}")
            nc.vector.tensor_scalar(
                h2, pa2s.pop(i), b2a if kind == "A" else b2b, 0.0, op0=add, op1=amax
            )
            h2s[i] = h2

        def stage5(i):
            kind = tiles[i][0]
            pa3 = pa3pool.tile([128, TILE], F32, tag="pa3", name=f"pa3_{i}")
            nc.tensor.matmul(
                pa3, w3a if kind == "A" else w3b, h2s.pop(i), start=True, stop=True
            )
            pa3s[i] = pa3

        def stage6(i):
            kind, s, c = tiles[i]
            if kind == "A":
                if c == 0:
                    outas[s] = oa_pool.tile(
                        [128, OUT_W], BF16, tag="oa", name=f"oa{s}"
                    )
                nc.scalar.activation(
                    outas[s][:, c * TILE : (c + 1) * TILE],
                    pa3s.pop(i),
                    ident,
                    bias=b3a,
                )
            else:
                nc.vector.tensor_scalar_add(
                    outas[s][:, SUPER:], pa3s.pop(i), b3b
                )

        def stage7(i):
            kind, s, c = tiles[i]
            if kind == "B":
                nc.sync.dma_start(outAB[s], outas.pop(s))

        in_dma(0)
        in_dma(1)
        stages = [stage7, stage6, stage5, stage4, stage3, stage2, stage1]
        for slot in range(NT + 6):
            for d, fn in enumerate(stages):
                i = slot - 6 + d
                if 0 <= i < NT:
                    fn(i)


def _make_in_maps(x, W1, b1, W2, b2, W3, b3):
    """Per-core input maps (host-side shard + pack)."""
    import ml_dtypes

    BF = ml_dtypes.bfloat16
    wp = _pack_weights(W1, b1, W2, b2, W3, b3)
    x3 = np.asarray(x, np.float32).reshape(N_CORES, SHARD, IN_DIM)
    in_maps = []
    for c in range(N_CORES):
        xT = np.zeros((128, PAD), BF)
        xT[:, :SHARD] = x3[c, :, :128].T
        xl = np.zeros((1, PAD), BF)
        xl[0, :SHARD] = x3[c, :, 128]
        xl4p = np.ascontiguousarray(
            xl.reshape(N_SUPERS, 4, TILE).transpose(1, 0, 2).reshape(4, PAD4)
        )
        m = {"xT": xT, "xl": xl, "xl4p": xl4p}
        m.update(wp)
        in_maps.append(m)
    return in_maps


_CACHE = {}


def _build(repeat=1):
    key = (repeat, os.environ.get("K_STRIP", ""), os.environ.get("K_NOXL", ""),
           os.environ.get("K_L23", ""))
    if key in _CACHE:
        return _CACHE[key]
    nc = bacc.Bacc(
        "TRN2",
        target_bir_lowering=False,
        debug=False,
        num_devices=N_CORES,
    )
    ins = {}
    L23D = F32R if os.environ.get("K_L23", "") == "f32r" else BF16
    ins["xT"] = nc.dram_tensor("xT", (128, PAD), BF16, kind="ExternalInput").ap()
    ins["xl"] = nc.dram_tensor("xl", (1, PAD), BF16, kind="ExternalInput").ap()
    for name, shape, dt in [
        ("w1a", (128, 128), BF16),
        ("wla", (1, 128), BF16),
        ("w1bs", (4, 128, 128), BF16),
        ("wlbs", (4, 1, 128), BF16),
        ("w2a", (128, 128), L23D),
        ("w2b", (128, 128), L23D),
        ("w3a", (128, 128), L23D),
        ("w3b", (128, 128), L23D),
        ("b1a", (128, 1), F32),
        ("b1b", (128, 1), F32),
        ("b2a", (128, 1), F32),
        ("b2b", (128, 1), F32),
        ("b3a", (128, 1), F32),
        ("b3b", (128, 1), F32),
    ]:
        ins[name] = nc.dram_tensor(name, shape, dt, kind="ExternalInput").ap()
    outs = {
        "outA": nc.dram_tensor(
            "outA", (N_SUPERS, 128, SUPER), BF16, kind="ExternalOutput"
        ).ap(),
        "outB": nc.dram_tensor(
            "outB", (N_SUPERS, 128, TILE), BF16, kind="ExternalOutput"
        ).ap(),
    }
    with tile.TileContext(nc) as tc:
        _kernel_body(tc, outs, ins, repeat=repeat)
    nc.compile()
    _CACHE[key] = nc
    return nc


def _decode_out(results):
    """Device layouts -> [10, BATCH, 16] f32."""
    out = np.empty((NHEADS, BATCH, SKIP), np.float32)
    for c in range(N_CORES):
        oa = np.asarray(results[c]["outA"])  # [31, 128, 2048] bf16
        ob = np.asarray(results[c]["outB"])  # [31, 128, 512] bf16
        # A: [s, (h, o), (j)] -> sample = s*2048 + j
        a = oa.reshape(N_SUPERS, 8, SKIP, SUPER).transpose(1, 0, 3, 2)
        a = a.reshape(8, PAD, SKIP)
        out[:8, c * SHARD : (c + 1) * SHARD] = a[:, :SHARD]
        # B: [s, (cc, g, o), b] -> sample = s*2048 + cc*512 + b
        b = ob.reshape(N_SUPERS, 4, 2, SKIP, TILE).transpose(2, 0, 1, 4, 3)
        b = b.reshape(2, PAD, SKIP)
        out[8:, c * SHARD : (c + 1) * SHARD] = b[:, :SHARD]
    return out


def kernel(x, W1, b1, W2, b2, W3, b3, _want_trace=False):
    in_maps = _make_in_maps(x, W1, b1, W2, b2, W3, b3)
    nc = _build()
    res = run_bass_kernel_spmd(
        nc, in_maps, core_ids=list(range(N_CORES)), trace=_want_trace
    )
    if _want_trace:
        kernel.last_results = res
    return _decode_out(res.results)
